# revision 5
# baseline (speedup 1.0000x reference)
"""Multi-head attention (4 heads, d_model=256, B=4, N=2048) on 8 trn2 cores.

Sharding: core c handles batch b=c//2, query-half j=c%2 (1024 queries).
No collectives; outputs are disjoint [256, 1024] slices.

Per-core pipeline (all matmuls float32r):
  Q = WqT.T @ xq + bq           (head-permuted rows, [64, 4, 1024])
  K = WkT.T @ xk                (bk dropped: constant-in-m, cancels in softmax)
  VT = xv.T @ WvT               ([m, 256] + ones column per head -> [128,16,4,65])
  per head h, key-chunk j: scoresT = K_hj.T @ Q_h   ([128 keys, 1024 queries])
     expT = exp(scoresT / 8)    (ACT, psum->sbuf f32r)
     pv  += [VT_hj | 1].T @ expT  -> psum [65, 1024]; row 64 = softmax denom
  att_h = pv[0:64] * (1/denom)  (recip replicated via K=1 ones matmul)
  out = WmT.T @ att             (bias bm + Wm@bv folded in on host)
"""
import sys

sys.path.insert(0, "/opt/trn_rl_repo")

import numpy as np
import concourse.bass as bass
import concourse.tile as tile
from concourse import bacc, mybir
from concourse.bass_utils import run_bass_kernel_spmd

F32 = mybir.dt.float32
F32R = mybir.dt.float32r
EXP = mybir.ActivationFunctionType.Exp
MULT = mybir.AluOpType.mult

NH, DM, HD = 4, 256, 64  # heads, d_model, head_dim
NQ = 1024  # queries per core
NK = 2048  # keys
NJ = NK // 128  # key chunks of 128


def build_nc():
    nc = bacc.Bacc("TRN2", target_bir_lowering=False, debug=False, num_devices=8)

    xq_d = nc.declare_dram_parameter("xq", [DM, NQ], F32R, isOutput=False)
    xk_d = nc.declare_dram_parameter("xk", [DM, NK], F32R, isOutput=False)
    xv_d = nc.declare_dram_parameter("xv", [DM, NK], F32R, isOutput=False)
    wq_d = nc.declare_dram_parameter("wq", [DM, DM], F32R, isOutput=False)
    wk_d = nc.declare_dram_parameter("wk", [DM, DM], F32R, isOutput=False)
    wv_d = nc.declare_dram_parameter("wv", [DM, DM], F32R, isOutput=False)
    wm_d = nc.declare_dram_parameter("wm", [DM, DM], F32R, isOutput=False)
    bq_d = nc.declare_dram_parameter("bq", [DM], F32, isOutput=False)
    out_d = nc.declare_dram_parameter("out", [DM, NQ], F32, isOutput=True)

    with tile.TileContext(nc) as tc:
        with (
            tc.tile_pool(name="persist", bufs=1) as persist,
            tc.tile_pool(name="work", bufs=3) as work,
            tc.tile_pool(name="pse", bufs=2, space="PSUM") as pse,
            tc.tile_pool(name="psv", bufs=2, space="PSUM") as psv,
        ):
            # ---- load inputs ----
            xq_t = persist.tile([128, 2, NQ], F32R)
            nc.sync.dma_start(out=xq_t, in_=xq_d.rearrange("(c p) n -> p c n", p=128))
            xk_t = persist.tile([128, 2, NK], F32R)
            nc.sync.dma_start(out=xk_t, in_=xk_d.rearrange("(c p) n -> p c n", p=128))
            xv_t = persist.tile([128, 2, NK], F32R)
            nc.sync.dma_start(out=xv_t, in_=xv_d.rearrange("(c p) n -> p c n", p=128))
            w_t = {}
            for name, d in (("wq", wq_d), ("wk", wk_d), ("wv", wv_d), ("wm", wm_d)):
                w_t[name] = persist.tile(
                    [128, 2, DM], F32R, tag=f"w_{name}", name=f"wt_{name}"
                )
                nc.sync.dma_start(
                    out=w_t[name], in_=d.rearrange("(c p) o -> p c o", p=128)
                )
            bq_t = persist.tile([128, 2], F32)
            nc.sync.dma_start(out=bq_t, in_=bq_d.rearrange("(c p) -> p c", p=128))
            ones32 = persist.tile([128, HD], F32, tag="ones32")
            nc.vector.memset(ones32, 1.0)
            ones_t = persist.tile([1, HD], F32R, tag="ones")
            nc.vector.tensor_copy(ones_t, ones32[0:1, :])

            # ---- Q projection -> q_t [64, NH, NQ] (head h rows at part 0-63) ----
            q_t = persist.tile([64, NH, NQ], F32R, tag="q")
            for oc in range(2):
                ps_q = pse.tile([128, NQ], F32, tag="big")
                for nt in range(2):
                    for ic in range(2):
                        nc.tensor.matmul(
                            ps_q[:, nt * 512 : (nt + 1) * 512],
                            w_t["wq"][:, ic, oc * 128 : (oc + 1) * 128],
                            xq_t[:, ic, nt * 512 : (nt + 1) * 512],
                            start=(ic == 0),
                            stop=(ic == 1),
                        )
                for s in range(2):
                    h = oc * 2 + s
                    nc.vector.tensor_scalar_add(
                        q_t[:, h, :],
                        ps_q[s * 64 : (s + 1) * 64, :],
                        bq_t[s * 64 : (s + 1) * 64, oc : oc + 1],
                    )

            # ---- K projection (no bias) -> k_t [64, NH, NK] ----
            k_t = persist.tile([64, NH, NK], F32R, tag="k")
            for oc in range(2):
                for half in range(2):
                    ps_k = pse.tile([128, NQ], F32, tag="big")
                    for nt in range(2):
                        for ic in range(2):
                            nc.tensor.matmul(
                                ps_k[:, nt * 512 : (nt + 1) * 512],
                                w_t["wk"][:, ic, oc * 128 : (oc + 1) * 128],
                                xk_t[
                                    :, ic, half * 1024 + nt * 512 : half * 1024 + (nt + 1) * 512
                                ],
                                start=(ic == 0),
                                stop=(ic == 1),
                            )
                    for s in range(2):
                        h = oc * 2 + s
                        nc.vector.tensor_copy(
                            k_t[:, h, half * 1024 : (half + 1) * 1024],
                            ps_k[s * 64 : (s + 1) * 64, :],
                        )

            # ---- V^T projection -> vt [128, NJ, NH, HD+1] (ones col appended) ----
            vt = persist.tile([128, NJ, NH, HD + 1], F32R, tag="vt")
            for j in range(NJ):
                ps_v = pse.tile([128, NQ], F32, tag="big")
                for ic in range(2):
                    nc.tensor.matmul(
                        ps_v[:, 0:DM],
                        xv_t[:, ic, j * 128 : (j + 1) * 128],
                        w_t["wv"][:, ic, :],
                        start=(ic == 0),
                        stop=(ic == 1),
                    )
                for h in range(NH):
                    nc.vector.tensor_copy(
                        vt[:, j, h, 0:HD], ps_v[:, h * HD : (h + 1) * HD]
                    )
                    nc.vector.tensor_copy(
                        vt[:, j, h, HD : HD + 1], ones32[:, 0:1]
                    )

            # ---- attention per head ----
            att_t = persist.tile([128, 2, NQ], F32R, tag="att")
            for h in range(NH):
                ps_o = psv.tile([HD + 1, NQ], F32, tag="pv")
                for j in range(NJ):
                    ps_s = pse.tile([128, NQ], F32, tag="big")
                    for nh in range(2):
                        nc.tensor.matmul(
                            ps_s[:, nh * 512 : (nh + 1) * 512],
                            k_t[:, h, j * 128 : (j + 1) * 128],
                            q_t[:, h, nh * 512 : (nh + 1) * 512],
                            start=True,
                            stop=True,
                        )
                    expt = work.tile([128, NQ], F32R, tag="exp")
                    nc.scalar.activation(expt, ps_s, EXP, scale=0.125)
                    for nh in range(2):
                        nc.tensor.matmul(
                            ps_o[:, nh * 512 : (nh + 1) * 512],
                            vt[:, j, h, :],
                            expt[:, nh * 512 : (nh + 1) * 512],
                            start=(j == 0),
                            stop=(j == NJ - 1),
                        )
                # normalize: att_h = ps_o[0:64] * (1/denom)
                rec32 = work.tile([1, NQ], F32, tag="rec32")
                nc.vector.reciprocal(rec32, ps_o[HD : HD + 1, :])
                rec_r = work.tile([1, NQ], F32R, tag="recr")
                nc.vector.tensor_copy(rec_r, rec32)
                ps_r = pse.tile([128, NQ], F32, tag="big")
                for nh in range(2):
                    nc.tensor.matmul(
                        ps_r[0:HD, nh * 512 : (nh + 1) * 512],
                        ones_t,
                        rec_r[:, nh * 512 : (nh + 1) * 512],
                        start=True,
                        stop=True,
                    )
                rec_rep = work.tile([HD, NQ], F32, tag="recrep")
                nc.vector.tensor_copy(rec_rep, ps_r[0:HD, :])
                nc.vector.tensor_tensor(
                    out=att_t[(h % 2) * 64 : (h % 2) * 64 + 64, h // 2, :],
                    in0=ps_o[0:HD, :],
                    in1=rec_rep,
                    op=MULT,
                )

            # ---- merge projection ----
            out_r = out_d.rearrange("(c p) n -> p c n", p=128)
            for oc in range(2):
                ps_m = pse.tile([128, NQ], F32, tag="big")
                for nt in range(2):
                    for ic in range(2):
                        nc.tensor.matmul(
                            ps_m[:, nt * 512 : (nt + 1) * 512],
                            w_t["wm"][:, ic, oc * 128 : (oc + 1) * 128],
                            att_t[:, ic, nt * 512 : (nt + 1) * 512],
                            start=(ic == 0),
                            stop=(ic == 1),
                        )
                out_t = work.tile([128, NQ], F32, tag="out")
                nc.vector.tensor_copy(out_t, ps_m)
                nc.sync.dma_start(out=out_r[:, oc, :], in_=out_t)

    nc.compile()
    return nc


_NC = None


def kernel(query, key, value, wq, bq, wk, bk, wv, bv, wm, bm):
    global _NC
    if _NC is None:
        _NC = build_nc()
    nc = _NC

    perm = np.array([(r % HD) * NH + r // HD for r in range(DM)])
    wqT = np.ascontiguousarray(wq[perm].T)
    wkT = np.ascontiguousarray(wk[perm].T)
    wvT = np.ascontiguousarray(wv[perm].T)
    wmT = np.ascontiguousarray(wm[:, perm].T)
    bqp = np.ascontiguousarray(bq[perm])
    bm_eff = (bm + wm @ bv).astype(np.float32)

    in_maps = []
    for c in range(8):
        b, j = c // 2, c % 2
        in_maps.append(
            {
                "xq": np.ascontiguousarray(query[b][:, j * NQ : (j + 1) * NQ]),
                "xk": np.ascontiguousarray(key[b]),
                "xv": np.ascontiguousarray(value[b]),
                "wq": wqT,
                "wk": wkT,
                "wv": wvT,
                "wm": wmT,
                "bq": bqp,
            }
        )

    res = run_bass_kernel_spmd(nc, in_maps, core_ids=list(range(8)))

    out = np.empty((4, DM, NK), np.float32)
    for c in range(8):
        b, j = c // 2, c % 2
        out[b][:, j * NQ : (j + 1) * NQ] = res.results[c]["out"] + bm_eff[:, None]
    return out
